# revision 57
# baseline (speedup 1.0000x reference)
"""Invariant Point Attention (IPA) forward as a Bass/Tile kernel on 8
Trainium2 NeuronCores (SPMD, query-axis sharded).

Strategy: each core owns 128 query rows of one batch. Attention logits are a
single 384-feature f32r matmul per 4-query block (QK, point-attention cross
term, and -0.5*hw*||kg||^2 folded into one contraction); the pair bias z@wb
accumulates directly into the logits PSUM via per-query bf16 matmuls. z is
shipped from the host in bf16 in TWO packed layouts — natural (j on
partitions, for out_pair) and pre-transposed (d on partitions, for the pair
bias) — so no on-chip transposes or PSUM-drain copies of z are needed and z
HBM traffic is halved.
"""
import sys
sys.path.insert(0, '/opt/trn_rl_repo')

import math
import numpy as np

import concourse.bass as bass
import concourse.tile as tile
from concourse import bacc, mybir
from concourse.masks import make_identity

C_S, C_Z = 384, 128
H, D = 12, 16
PQ, PV = 4, 8
B, N = 2, 512
OUT_IN = H * D + H * PV * 4 + H * C_Z  # 2112
EPS = 1e-8

N_CORES = 8
NI = 128            # query rows per core
IBLK = 4            # queries per block (each padded to 32 partitions)
NBLK = NI // IBLK   # 32 blocks
SBLK = 8            # blocks per superblock (32 i's)
NSUP = NBLK // SBLK  # 4 superblocks

SCALE_QK = math.sqrt(1.0 / (3.0 * D))
SCALE_B = math.sqrt(1.0 / 3.0)
WC = math.sqrt(1.0 / (3.0 * (PQ * 9.0 / 2.0)))

# ---- proj feature column order for the fused projection s @ Wcat ----
#   A 0:192     k    (h*16+dd)
#   B 192:384   v    (h*16+dd)
#   C 384:528   kp   (c*48 + h*4 + q)
#   D 528:816   vp   (c*96 + h*8 + pv)
#   E 816:1008  q    (h*16+dd)
#   F 1008:1152 qp   (c*48 + h*4 + q)
PROJ = 1152
# host packing offsets (pre-permutation)
OFF_K, OFF_V, OFF_KP, OFF_VP, OFF_Q, OFF_QP = 0, 192, 384, 528, 816, 1008
# device column order after permutation: K | KP | V | VP | Q | QP
DOFF_K, DOFF_KP, DOFF_V, DOFF_VP, DOFF_Q, DOFF_QP = 0, 192, 336, 528, 816, 1008

# kfullT / fT_q global row map (384 rows = 3 chunks of 128):
#   0:192    k rows   (h*16+dd)      [chunk0 0:128 | chunk1 0:64]
#   192:336  kg rows  (c*48+h*4+q)   [chunk1 64:128 | chunk2 0:64 | chunk2 64:80]
#   352:364  C / E rows (h)          [chunk2 96:108]
KF_PAD = 384


def softplus(x):
    return np.logaddexp(0.0, x)


def pack_weights(inp):
    """Host-side packing of the small weight tensors (layout-only + tiny
    scalar math). Returns dict of arrays shared by all cores."""
    wq, bq = np.asarray(inp['wq']), np.asarray(inp['bq'])
    wkv, bkv = np.asarray(inp['wkv']), np.asarray(inp['bkv'])
    wqp, bqp = np.asarray(inp['wqp']), np.asarray(inp['bqp'])
    wkvp, bkvp = np.asarray(inp['wkvp']), np.asarray(inp['bkvp'])
    wb = np.asarray(inp['wb'])
    wout, bout = np.asarray(inp['wout']), np.asarray(inp['bout'])
    head_weights = np.asarray(inp['head_weights'])

    wcat = np.zeros((C_S, PROJ), np.float32)
    bcat = np.zeros((PROJ,), np.float32)

    kv = wkv.reshape(C_S, H, 2 * D)
    bkv2 = bkv.reshape(H, 2 * D)
    wcat[:, OFF_K:OFF_K + 192] = kv[:, :, :D].reshape(C_S, 192)
    bcat[OFF_K:OFF_K + 192] = bkv2[:, :D].reshape(192)
    wcat[:, OFF_V:OFF_V + 192] = kv[:, :, D:].reshape(C_S, 192)
    bcat[OFF_V:OFF_V + 192] = bkv2[:, D:].reshape(192)

    kvp = wkvp.reshape(C_S, 3, H, PQ + PV)
    bkvp2 = bkvp.reshape(3, H, PQ + PV)
    wcat[:, OFF_KP:OFF_KP + 144] = kvp[:, :, :, :PQ].reshape(C_S, 144)
    bcat[OFF_KP:OFF_KP + 144] = bkvp2[:, :, :PQ].reshape(144)
    wcat[:, OFF_VP:OFF_VP + 288] = kvp[:, :, :, PQ:].reshape(C_S, 288)
    bcat[OFF_VP:OFF_VP + 288] = bkvp2[:, :, PQ:].reshape(288)
    # device order: K | KP | V | VP | Q | QP  (V+VP contiguous -> one
    # proj section draining to a persistent tile, transformed in-loop)
    perm_cols = (list(range(0, 192)) + list(range(384, 528))
                 + list(range(192, 384)) + list(range(528, 816))
                 + list(range(816, 1152)))
    wcat = wcat[:, perm_cols]
    bcat = bcat[perm_cols]

    wcat[:, OFF_Q:OFF_Q + 192] = wq
    bcat[OFF_Q:OFF_Q + 192] = bq
    wcat[:, OFF_QP:OFF_QP + 144] = wqp
    bcat[OFF_QP:OFF_QP + 144] = bqp

    wcat_dev = wcat.reshape(3, 128, PROJ).copy()

    hw = (softplus(head_weights) * WC).astype(np.float32)          # [H]

    # wbs: wb * SCALE_B (bb is constant over j -> softmax-invariant, dropped)
    wbs = (wb * SCALE_B).astype(np.float32)

    # qsel [3, 128, H]: row-selector+scale for building qexp from fT_q.
    qsel = np.zeros((3, 128, H), np.float32)
    for h in range(H):
        for dd in range(D):
            g = h * D + dd           # k rows
            qsel[g // 128, g % 128, h] = SCALE_QK
    for c in range(3):
        for h in range(H):
            for q in range(PQ):
                g = c * 48 + h * 4 + q   # kg feature index 0..144
                if g < 64:
                    ch, r = 1, 64 + g
                elif g < 128:
                    ch, r = 2, g - 64
                else:
                    ch, r = 2, 64 + (g - 128)
                qsel[ch, r, h] = hw[h]
    for h in range(H):
        qsel[2, 96 + h, h] = -0.5 * hw[h]   # E rows

    # wout reordered+padded to featsT row order, 18 chunks of 128 (bf16):
    import ml_dtypes
    wout_dev = np.zeros((18, 128, C_S), ml_dtypes.bfloat16)
    wo = wout.astype(ml_dtypes.bfloat16)  # [2112, 384]
    wout_dev[0] = wo[0:128]
    wout_dev[1, :64] = wo[128:192]
    wout_dev[2, :96] = wo[192:288]      # rpl_x
    wout_dev[3, :96] = wo[288:384]      # rpl_y
    wout_dev[4, :96] = wo[384:480]      # rpl_z
    wout_dev[5, :96] = wo[480:576]      # norms
    for h in range(H):
        wout_dev[6 + h] = wo[576 + h * C_Z: 576 + (h + 1) * C_Z]

    return dict(
        wcat_dev=wcat_dev, bcat=bcat.reshape(1, PROJ), wbs=wbs,
        qsel=qsel, wout_dev=wout_dev,
        bout=bout.reshape(1, C_S).astype(np.float32), hw=hw,
    )


def per_core_inputs(inp, packed, core):
    """Build the input map for one core.

    The j (key/residue) axis is rolled per core so this core's 128 query
    rows sit at j'=0..127 — softmax/attention sums over j are order
    invariant, and the query-side projections/frames become the jc=0 slice
    of the key-side ones (no separate sTq/Rtq stream needed).
    """
    b = core // 4
    i0 = (core % 4) * NI
    s = np.asarray(inp['single_representation'])[b]          # [512, 384]
    z = np.asarray(inp['pair_representation'])[b]            # [512, 512, 128]
    R = np.asarray(inp['rotations'])[b]                      # [512, 3, 3]
    t = np.asarray(inp['translation'])[b]                    # [512, 3]

    perm = (np.arange(N) + i0) % N
    s = s[perm]
    R = R[perm]
    t = t[perm]

    sT = np.ascontiguousarray(s.T.reshape(3, 128, N))
    Rt = np.concatenate([R.reshape(N, 9), t], axis=1).astype(np.float32)  # [512, 12]

    # z slab for our 128 queries (j rolled), bf16, two packed layouts:
    #   znat[bk, p, il, jc, d] = z'[bk*4+il, jc*128+p, d]   (j on partitions)
    #   ztr [bk, d, il, j]     = z'[bk*4+il, j, d]          (d on partitions)
    import ml_dtypes
    zq = z[i0:i0 + NI][:, perm].astype(ml_dtypes.bfloat16)   # [128, 512, 128]
    z5 = zq.reshape(NBLK, IBLK, 4, 128, C_Z)                 # [bk, il, jc, p, d]
    znat = np.ascontiguousarray(z5.transpose(0, 3, 1, 2, 4)).reshape(NBLK, 128, IBLK * 4 * C_Z)
    # bias path is softmax-smoothed: fp8 is enough and halves its HBM bytes
    zq8 = z[i0:i0 + NI][:, perm].astype(ml_dtypes.float8_e4m3fn)
    ztr = np.ascontiguousarray(zq8.reshape(NBLK, IBLK, N, C_Z).transpose(0, 3, 1, 2)).reshape(NBLK, C_Z, IBLK * N)

    return {
        'sT': sT,
        'znat': znat, 'ztr': ztr,
        'Rt': np.ascontiguousarray(Rt),
        'wcat_dev': packed['wcat_dev'], 'bcat': packed['bcat'],
        'wbs': packed['wbs'], 'qsel': packed['qsel'],
        'wout_dev': packed['wout_dev'], 'bout': packed['bout'],
    }, b, i0


F32 = mybir.dt.float32
F32R = mybir.dt.float32r
BF16 = mybir.dt.bfloat16
F8E4 = mybir.dt.float8e4
AF = mybir.ActivationFunctionType
ALU = mybir.AluOpType
AX = mybir.AxisListType


def r32(x):
    return x.bitcast(F32R)


def declare_params(nc):
    p = {}
    p['sT'] = nc.declare_dram_parameter("sT", [3, 128, N], F32R, isOutput=False)
    p['znat'] = nc.declare_dram_parameter("znat", [NBLK, 128, IBLK * 4 * C_Z], BF16, isOutput=False)
    p['ztr'] = nc.declare_dram_parameter("ztr", [NBLK, C_Z, IBLK * N], F8E4, isOutput=False)
    p['Rt'] = nc.declare_dram_parameter("Rt", [N, 12], F32, isOutput=False)
    p['wcat'] = nc.declare_dram_parameter("wcat_dev", [3, 128, PROJ], F32R, isOutput=False)
    p['bcat'] = nc.declare_dram_parameter("bcat", [1, PROJ], F32R, isOutput=False)
    p['wbs'] = nc.declare_dram_parameter("wbs", [C_Z, H], F32, isOutput=False)
    p['qsel'] = nc.declare_dram_parameter("qsel", [3, 128, H], F32, isOutput=False)
    p['wout'] = nc.declare_dram_parameter("wout_dev", [18, 128, C_S], BF16, isOutput=False)
    p['bout'] = nc.declare_dram_parameter("bout", [1, C_S], F32R, isOutput=False)
    p['out'] = nc.declare_dram_parameter("out", [NI, C_S], F32, isOutput=True)
    return p


def build_kernel(repeat=1):
    nc = bacc.Bacc("TRN2", target_bir_lowering=False, debug=False,
                   num_devices=N_CORES)
    p = declare_params(nc)

    with tile.TileContext(nc) as tc:
        if repeat > 1:
            with tc.For_i(0, repeat, 1):
                _body(nc, tc, p)
        else:
            _body(nc, tc, p)
    nc.compile()
    return nc


def _body(nc, tc, p):
    dma = nc.sync.dma_start

    pers_cm = tc.tile_pool(name="pers", bufs=1)
    pers = pers_cm.__enter__()

    ident = pers.tile([128, 128], F32)
    make_identity(nc, ident[:])
    ident_r = pers.tile([128, 128], F32R)
    nc.vector.tensor_copy(ident_r[:], ident[:])
    ident_b = pers.tile([128, 128], BF16)
    nc.vector.tensor_copy(ident_b[:], ident[:])
    ones1 = pers.tile([1, 128], F32R)
    nc.gpsimd.memset(ones1[:].bitcast(F32), 1.0)

    wbs_bf = pers.tile([C_Z, H], F8E4)
    Rt_sb = pers.tile([128, 4 * 12], F32)
    dma(Rt_sb[:].rearrange("p (jc w) -> p jc w", jc=4),
        p['Rt'].rearrange("(jc p) w -> p jc w", jc=4))

    kfullT = [pers.tile([128, N], F32R, tag=f"kfullT{c}", name=f"kfullT{c}")
              for c in range(3)]
    qexp = [pers.tile([128, NI * 32], F32R, tag=f"qexp{c}", name=f"qexp{c}")
            for c in range(3)]
    # Only rows that stay junk after the phase-0 writes need zeroing (junk
    # could be NaN bit patterns; NaN*0 = NaN would poison the logits).
    nc.gpsimd.memset(kfullT[2][64:128, :].bitcast(F32), 0.0)
    vvg_bf = pers.tile([128, 4 * 480], BF16)
    pairT = pers.tile([128, H * 128], BF16)
    fts = pers.tile([128, 6 * 128], BF16)
    # fts rows never written per-chunk must be zero (chunk1 rows 64:128,
    # chunks 2..5 rows 96:128) so junk doesn't poison the final projection.
    nc.gpsimd.memset(fts[64:128, 128:256], 0.0)
    nc.gpsimd.memset(fts[96:128, 256:768], 0.0)
    sumsq = pers.tile([128, 96], F32)
    vraw = pers.tile([128, 4 * 480], F32)   # V|VP proj drains, per j-chunk
    wout_sb = pers.tile([128, 18 * C_S], BF16)
    bout_sb = pers.tile([1, C_S], F32R)
    epsb = pers.tile([128, 1], F32)
    nc.gpsimd.memset(epsb[:], EPS)

    # z streaming pools opened before phase-0 scratch so their SBUF does not
    # alias phase-0 tiles — z DMAs then overlap phase-0 compute. z_sb gets
    # extra depth: the deferred out_pair stage keeps it alive one block
    # longer than zt_sb.
    zpn_cm = tc.tile_pool(name="zpn", bufs=6)
    zpn = zpn_cm.__enter__()
    zpt_cm = tc.tile_pool(name="zpt", bufs=6)
    zpt = zpt_cm.__enter__()

    PREFETCH = 6
    z_tiles = {}

    def fetch_block(bk):
        z_sb = zpn.tile([128, IBLK * 4 * C_Z], BF16, tag="z", name="z_sb")
        dma(z_sb[:], p['znat'][bk])
        zt_sb = zpt.tile([C_Z, IBLK * N], F8E4, tag="zt", name="zt_sb")
        dma(zt_sb[:], p['ztr'][bk])
        z_tiles[bk] = (z_sb, zt_sb)

    # =================== PHASE 0 ===================
    with tc.tile_pool(name="ph0", bufs=1) as ph0, \
         tc.tile_pool(name="ph0b", bufs=2) as ph0b, \
         tc.tile_pool(name="ph0ps", bufs=2, space="PSUM") as ph0ps:
        # Weight/input DMAs split per chunk and interleaved so the first
        # proj matmuls can start as soon as chunk 0 lands.
        wcat_sb = ph0.tile([128, 3 * PROJ], F32R)
        sT_sb = ph0.tile([128, 3 * N], F32R)
        for kc in range(3):
            dma(wcat_sb[:, kc * PROJ:(kc + 1) * PROJ], p['wcat'][kc])
            dma(sT_sb[:, kc * N:(kc + 1) * N], p['sT'][kc])
        bcat_sb = ph0.tile([1, PROJ], F32R)
        dma(bcat_sb[:], p['bcat'][:])
        wbs_f = ph0.tile([C_Z, H], F32)
        dma(wbs_f[:], p['wbs'][:])
        nc.vector.tensor_copy(wbs_bf[:], wbs_f[:])
        qsel_sb = ph0.tile([128, 3 * H], F32)
        dma(qsel_sb[:].rearrange("p (c w) -> p c w", c=3),
            p['qsel'].rearrange("c p w -> p c w"))
        dma(bout_sb[:], p['bout'][:])

        # First blocks' z DMAs: after the phase-0 weight DMAs (which gate
        # compute) but before the phase-0 closing barrier, so they stream
        # while phase-0 computes.
        for bk in range(PREFETCH):
            fetch_block(bk)

        proj_nat = ph0.tile([128, 4 * PROJ], F32R)   # j-chunks; jc=0 = queries
        projq = proj_nat[:, 0:PROJ]
        kg_nat = ph0.tile([128, 4 * 144], F32R)
        qg_nat = ph0.tile([128, 144], F32R)
        c_nat = ph0.tile([128, 4 * 12], F32R)
        fTq = [ph0.tile([128, NI], F32R, tag=f"fTq{c}", name=f"fTq{c}")
               for c in range(3)]
        nc.gpsimd.memset(fTq[2][64:128, :].bitcast(F32), 0.0)
        nc.gpsimd.memset(fTq[2][96:108, :].bitcast(F32), 1.0)

        proj_rr = [0]

        def proj_jc(jc):
            # sections: S0 K|KP -> proj_nat, S1 V|VP -> persistent vraw,
            # S2 Q|QP (queries chunk only) -> proj_nat
            secs = [(0, 336, 'p'), (336, 816, 'v')]
            if jc == 0:
                secs.append((816, 1152, 'p'))
            for (w0, w1, kind) in secs:
                wd = w1 - w0
                pp = ph0ps.tile([128, 480], F32, tag="projps", name="pp")
                for kc in range(3):
                    nc.tensor.matmul(
                        pp[:, 0:wd],
                        r32(sT_sb[:, kc * N + jc * 128: kc * N + (jc + 1) * 128]),
                        r32(wcat_sb[:, kc * PROJ + w0: kc * PROJ + w1]),
                        start=(kc == 0), stop=False)
                nc.tensor.matmul(pp[:, 0:wd], ones1[:], bcat_sb[:, w0:w1],
                                 start=False, stop=True)
                eng = proj_rr[0] % 2
                proj_rr[0] += 1
                if kind == 'v':
                    dst = vraw[:, jc * 480:(jc + 1) * 480]
                else:
                    dcol = jc * PROJ + w0
                    dst = proj_nat[:, dcol:dcol + wd]
                if eng == 0:
                    nc.vector.tensor_copy(dst, pp[:, 0:wd])
                else:
                    nc.scalar.copy(dst, pp[:, 0:wd])

        def glob_points(dst, dcol0, src, scol0, bw, n_rt, eng=None):
            eng = eng or nc.vector
            for cp in range(3):
                d = dst[:, dcol0 + cp * bw: dcol0 + (cp + 1) * bw]
                eng.tensor_scalar(
                    d, src[:, scol0:scol0 + bw].bitcast(F32),
                    Rt_sb[:, n_rt + cp * 3: n_rt + cp * 3 + 1],
                    Rt_sb[:, n_rt + 9 + cp: n_rt + 9 + cp + 1],
                    op0=ALU.mult, op1=ALU.add)
                for pp_ in (1, 2):
                    eng.scalar_tensor_tensor(
                        d, src[:, scol0 + pp_ * bw: scol0 + (pp_ + 1) * bw].bitcast(F32),
                        Rt_sb[:, n_rt + cp * 3 + pp_: n_rt + cp * 3 + pp_ + 1],
                        d.bitcast(F32),
                        op0=ALU.mult, op1=ALU.add)

        def cterm_jc(jc):
            sq = ph0b.tile([128, 144], F32, tag="sq", name="sq")
            nc.scalar.activation(sq[:], kg_nat[:, jc * 144:(jc + 1) * 144].bitcast(F32), AF.Square)
            red1 = ph0b.tile([128, 12 * 3], F32, tag="red1", name="red1")
            nc.vector.tensor_reduce(
                red1[:].rearrange("p (h c) -> p h c", h=H),
                sq[:].rearrange("p (c h q) -> p h c q", c=3, h=H),
                axis=AX.X, op=ALU.add)
            with nc.allow_low_precision(reason="f32r rounding of Cterm is fine"):
                nc.vector.tensor_reduce(
                    c_nat[:, jc * 12:(jc + 1) * 12].unsqueeze(-1),
                    red1[:].rearrange("p (h c) -> p h c", h=H),
                    axis=AX.X, op=ALU.add)

        # PSUM drains round-robin DVE/Pool/Act — phase 0 is DVE-bound.
        pe_T_rr = [0]

        def pe_T(dst_col_writes, src_ap):
            tp = ph0ps.tile([128, 128], F32R, tag="tps", name="tp")
            np_ = src_ap.shape[0]
            nf = src_ap.shape[1]
            nc.tensor.transpose(tp[0:nf, 0:np_], r32(src_ap),
                                ident_r[0:np_, 0:np_])
            for (dst, r0, r1) in dst_col_writes:
                eng = pe_T_rr[0] % 2
                pe_T_rr[0] += 1
                if r0 % 64 != 0 or eng:
                    nc.scalar.copy(dst, tp[r0:r1, 0:np_].bitcast(F32))
                else:
                    nc.vector.tensor_copy(dst, tp[r0:r1, 0:np_].bitcast(F32))

        def transposes_jc(jc):
            js = slice(jc * 128, (jc + 1) * 128)
            pe_T([(kfullT[0][0:128, js], 0, 128)],
                 proj_nat[:, jc * PROJ + DOFF_K: jc * PROJ + DOFF_K + 128])
            pe_T([(kfullT[1][0:64, js], 0, 64)],
                 proj_nat[:, jc * PROJ + DOFF_K + 128: jc * PROJ + DOFF_K + 192])
            pe_T([(kfullT[1][64:128, js], 0, 64), (kfullT[2][0:64, js], 64, 128)],
                 kg_nat[:, jc * 144: jc * 144 + 128])
            pe_T([(kfullT[2][64:80, js], 0, 16)],
                 kg_nat[:, jc * 144 + 128: jc * 144 + 144])
            pe_T([(kfullT[2][96:108, js], 0, 12)], c_nat[:, jc * 12:(jc + 1) * 12])

        # Critical-path order: everything the first block needs (proj, kg,
        # Cterm, kfullT, fTq, qexp), then the epilogue-only vg/vvg.
        for jc in range(4):
            proj_jc(jc)
            glob_points(kg_nat, jc * 144, proj_nat, jc * PROJ + DOFF_KP, 48,
                        jc * 12)
            cterm_jc(jc)
            if jc == 0:
                glob_points(qg_nat, 0, projq, DOFF_QP, 48, 0)
            transposes_jc(jc)
            if jc == 0:
                pe_T([(fTq[0][0:128, :], 0, 128)], projq[:, DOFF_Q:DOFF_Q + 128])
                pe_T([(fTq[1][0:64, :], 0, 64)], projq[:, DOFF_Q + 128:DOFF_Q + 192])
                pe_T([(fTq[1][64:128, :], 0, 64), (fTq[2][0:64, :], 64, 128)],
                     qg_nat[:, 0:128])
                pe_T([(fTq[2][64:80, :], 0, 16)], qg_nat[:, 128:144])
                # qexp: per-head masked scale of fT_q. Only e<12 columns are
                # written; e>=12 stay junk — junk lg rows exp to junk attn
                # rows/attnT cols that nothing downstream reads.
                for c in range(3):
                    for h in range(H):
                        dst = qexp[c][:].rearrange("p (i e) -> p i e", e=32)[:, :, h]
                        args = (fTq[c][:].bitcast(F32),
                                qsel_sb[:, c * H + h: c * H + h + 1])
                        if h % 2 == 0:
                            nc.vector.tensor_scalar(dst, args[0], args[1], None,
                                                    op0=ALU.mult)
                        else:
                            nc.scalar.mul(dst, args[0], args[1])


    # =================== MAIN LOOP ===================
    # Software-pipelined one block deep: block bk's post-softmax stage
    # (attnT, PSUM drains, out_pair) is emitted during block bk+1, after
    # bk+1's logits matmuls — so the in-order PE queue never stalls on the
    # softmax chain with useful work queued behind it.
    with tc.tile_pool(name="blk", bufs=2) as blkp, \
         tc.tile_pool(name="sblk", bufs=2) as sblkp, \
         tc.tile_pool(name="ps_lg", bufs=2, space="PSUM") as ps_lg, \
         tc.tile_pool(name="ps_at", bufs=1, space="PSUM") as ps_at, \
         tc.tile_pool(name="ps_op", bufs=1, space="PSUM") as ps_op, \
         tc.tile_pool(name="ps_ep", bufs=1, space="PSUM") as ps_ep, \
         tc.tile_pool(name="ps_fin", bufs=1, space="PSUM") as ps_fin:
        attnTs = {}
        deferred = {}

        def vvg_build():
            # V|VP -> global-frame vg -> vvg_bf, off the phase-0 barrier:
            # runs during the first blocks, needed first at sb 0's epilogue.
            for jc in range(4):
                vg = blkp.tile([128, 288], F32, tag="vg", name="vg")
                glob_points(vg, 0, vraw, jc * 480 + 192, 96, jc * 12)
                nc.scalar.copy(
                    vvg_bf[:, jc * 480:(jc + 1) * 480]
                    .rearrange("p (h w) -> p h w", h=H)[:, :, 0:D],
                    vraw[:, jc * 480: jc * 480 + 192]
                    .rearrange("p (h d) -> p h d", h=H))
                nc.vector.tensor_copy(
                    vvg_bf[:, jc * 480:(jc + 1) * 480]
                    .rearrange("p (h w) -> p h w", h=H)[:, :, D:D + 24]
                    .rearrange("p h (c v) -> p c h v", c=3),
                    vg[:].rearrange("p (c h v) -> p c h v", c=3, h=H))

        def front_stage(bk):
            sb, lb = bk // SBLK, bk % SBLK
            if lb == 0:
                attnTs[sb] = sblkp.tile([128, SBLK * 4 * 128], BF16,
                                        tag="attnTb", name="attnT_b")
            z_sb, zt_sb = z_tiles.pop(bk)
            if bk + PREFETCH < NBLK:
                fetch_block(bk + PREFETCH)
            # wout is epilogue-only: stream it mid-loop in 3 slices so it
            # neither delays the first z blocks nor stalls the tail.
            if bk in (2, 6, 10):
                c0 = {2: 0, 6: 6, 10: 12}[bk]
                dma(wout_sb[:, c0 * C_S:(c0 + 6) * C_S]
                    .rearrange("p (c w) -> p c w", c=6),
                    p['wout'][c0:c0 + 6].rearrange("c p w -> p c w"))

            lg = ps_lg.tile([128, N], F32, tag="lg", name="lg")
            for kc in range(3):
                nc.tensor.matmul(
                    lg[:], r32(qexp[kc][:, bk * 128:(bk + 1) * 128]),
                    r32(kfullT[kc][:]), start=(kc == 0), stop=(kc == 2))
            for il in range(IBLK):
                nc.tensor.matmul(lg[32 * il:32 * il + H, :], wbs_bf[:],
                                 zt_sb[:, il * N:(il + 1) * N],
                                 start=False, stop=True,
                                 skip_group_check=True,
                                 tile_position=(0, 32 * il))

            # softmax (exp + sum + normalize), bf16 weights
            attn = blkp.tile([128, N], BF16, tag="attn", name="attn")
            ssum = blkp.tile([128, 1], F32, tag="ssum", name="ssum")
            nc.scalar.activation(attn[:], lg[:], AF.Exp, accum_out=ssum[:])
            rcp = blkp.tile([128, 1], F32, tag="rcp", name="rcp")
            nc.vector.reciprocal(rcp[:], ssum[:])
            nc.vector.tensor_scalar(attn[:], attn[:], rcp[:], None,
                                    op0=ALU.mult)
            if bk == NBLK - 1:
                # Anchor a tiny Sqrt right after the last Exp so the sqrt
                # act-table load overlaps the drain instead of the tail.
                warm = blkp.tile([1, 1], F32, tag="warm", name="warm")
                nc.scalar.activation(warm[:], ssum[0:1, :], AF.Sqrt)
            deferred[bk] = (z_sb, attn)

        def back_stage(bk):
            sb, lb = bk // SBLK, bk % SBLK
            z_sb, attn = deferred.pop(bk)
            attnT_b = attnTs[sb]
            # attnT -> bf16 slab (epilogue lhsT + out_pair rhs);
            # PSUM drain split Pool/DVE so neither is on the whole chain.
            atp = ps_at.tile([128, N], BF16, tag="atp", name="atp")
            for jc in range(4):
                nc.tensor.transpose(atp[:, jc * 128:(jc + 1) * 128],
                                    attn[:, jc * 128:(jc + 1) * 128],
                                    ident_b[:])
            at_dst = attnT_b[:].rearrange("p (jc l) -> p jc l", jc=4)
            nc.vector.tensor_copy(
                at_dst[:, :, lb * 128:(lb + 1) * 128],
                atp[:].rearrange("p (jc e) -> p jc e", jc=4))

            # out_pair, all 4 queries batched into one PSUM tile + 1 copy
            op_ps = ps_op.tile([128, IBLK * H], F32, tag="opps", name="op_ps")
            for il in range(IBLK):
                for jc in range(4):
                    nc.tensor.matmul(
                        op_ps[:, il * H:(il + 1) * H],
                        z_sb[:, (il * 4 + jc) * C_Z:(il * 4 + jc + 1) * C_Z],
                        attnT_b[:, jc * (SBLK * 128) + lb * 128 + il * 32:
                                jc * (SBLK * 128) + lb * 128 + il * 32 + H],
                        start=(jc == 0), stop=(jc == 3),
                        skip_group_check=True)
            dst = (pairT[:].rearrange("p (h i) -> p h i", h=H)
                   [:, :, bk * IBLK:(bk + 1) * IBLK])
            src = op_ps[:].rearrange("p (il h) -> p h il", il=IBLK)
            if lb % 2 == 0:
                nc.vector.tensor_copy(dst, src)
            else:
                nc.scalar.copy(dst, src)

        def sb_epilogue(sb):
            attnT_b = attnTs.pop(sb)
            r0 = sb * 32
            epi = ps_ep.tile([32, H * 40], F32, tag="epi", name="epi")
            at4 = attnT_b[:].rearrange("p (jc gil e) -> p jc gil e",
                                       jc=4, e=32)
            for h in range(H):
                for jc in range(4):
                    nc.tensor.matmul(
                        epi[:, h * 40:(h + 1) * 40],
                        at4[:, jc, :, h],
                        vvg_bf[:, jc * 480 + h * 40: jc * 480 + (h + 1) * 40],
                        start=(jc == 0), stop=(jc == 3))
            scal_sb = blkp.tile([32, H * D], F32R, tag="scal", name="scal_sb")
            rpg_sb = blkp.tile([32, 3 * 96], F32, tag="rpg", name="rpg_sb")
            nc.scalar.copy(
                scal_sb[:].rearrange("p (h d) -> p h d", h=H),
                epi[:].rearrange("p (h w) -> p h w", h=H)[:, :, 0:D])
            nc.vector.tensor_copy(
                rpg_sb[:].rearrange("p (c h v) -> p c h v", c=3, h=H),
                epi[:].rearrange("p (h w) -> p h w", h=H)[:, :, D:40]
                .rearrange("p h (c v) -> p c h v", c=3))

            # ---- rpl rotation + sumsq + feature transposes, spread per sb ----
            rows = slice(r0, r0 + 32)
            # Frame scalars restaged at partition base 0 (scalar-ptr operands
            # must share the base partition of the tensor operand).
            rtq32 = blkp.tile([32, 12], F32, tag="rtq32", name="rtq32")
            nc.scalar.copy(rtq32[:], Rt_sb[rows, 0:12])
            rpgm = blkp.tile([32, 3 * 96], F32, tag="rpgm", name="rpgm")
            for pp_ in range(3):
                nc.vector.tensor_scalar(rpgm[:, pp_ * 96:(pp_ + 1) * 96],
                                        rpg_sb[:, pp_ * 96:(pp_ + 1) * 96],
                                        rtq32[:, 9 + pp_: 9 + pp_ + 1], None,
                                        op0=ALU.subtract)
            rpl = blkp.tile([32, 3 * 96], F32R, tag="rpl", name="rpl")
            for o in range(3):
                eng = nc.vector
                d = rpl[:, o * 96:(o + 1) * 96]
                eng.tensor_scalar(d, rpgm[:, 0:96], rtq32[:, o:o + 1], None,
                                  op0=ALU.mult)
                for pp_ in (1, 2):
                    eng.scalar_tensor_tensor(
                        d, rpgm[:, pp_ * 96:(pp_ + 1) * 96],
                        rtq32[:, pp_ * 3 + o: pp_ * 3 + o + 1], d.bitcast(F32),
                        op0=ALU.mult, op1=ALU.add)
            sq2 = blkp.tile([32, 3 * 96], F32, tag="sq2", name="sq2")
            nc.scalar.activation(sq2[:], rpl[:].bitcast(F32), AF.Square)
            nsq = blkp.tile([32, 96], F32, tag="nsq", name="nsq")
            nc.vector.tensor_tensor(nsq[:], sq2[:, 0:96], sq2[:, 96:192], op=ALU.add)
            nc.vector.tensor_tensor(nsq[:], nsq[:], sq2[:, 192:288], op=ALU.add)
            nc.scalar.copy(sumsq[rows, :], nsq[:])

            t2rr = [0]

            def pe_T2(dst, src_ap, nrows):
                tp2 = ps_ep.tile([128, 32], F32R, tag="tps2", name="tp2")
                nc.tensor.transpose(tp2[0:nrows, :], r32(src_ap), ident_r[0:32, 0:32])
                t2rr[0] += 1
                if t2rr[0] % 2:
                    nc.vector.tensor_copy(dst, tp2[0:nrows, :].bitcast(F32))
                else:
                    nc.scalar.copy(dst, tp2[0:nrows, :].bitcast(F32))

            pe_T2(fts[0:128, r0:r0 + 32], scal_sb[:, 0:128], 128)
            pe_T2(fts[0:64, 128 + r0:128 + r0 + 32], scal_sb[:, 128:192], 64)
            for o in range(3):
                pe_T2(fts[0:96, (2 + o) * 128 + r0:(2 + o) * 128 + r0 + 32],
                      rpl[:, o * 96:(o + 1) * 96], 96)

            if sb == NSUP - 1:
                # norms = sqrt(sumsq + eps); Sqrt switches the act table once,
                # after the last Exp.
                nrm = blkp.tile([128, 96], F32R, tag="nrm", name="nrm")
                nc.scalar.activation(nrm[:], sumsq[:], AF.Sqrt, bias=epsb[:])
                tpn = ps_ep.tile([128, 128], F32R, tag="tpn", name="tpn")
                nc.tensor.transpose(tpn[0:96, :], nrm[:], ident_r[:])
                nc.vector.tensor_copy(fts[0:96, 5 * 128:6 * 128],
                                      tpn[0:96, :].bitcast(F32))

        for bk in range(NBLK):
            front_stage(bk)
            if bk == 1:
                vvg_build()
            if bk > 0:
                back_stage(bk - 1)
            if bk % SBLK == 0 and bk > 0:
                sb_epilogue(bk // SBLK - 1)
        back_stage(NBLK - 1)
        sb_epilogue(NSUP - 1)

        # ---- final projection, inside the pool scope (no close barrier) ----
        fin = ps_fin.tile([128, C_S], F32, tag="fin", name="fin")
        fin_order = list(range(6, 18)) + [0, 1, 2, 3, 4, 5]
        for idx, c in enumerate(fin_order):
            lhsT = fts[:, c * 128:(c + 1) * 128] if c < 6 else \
                pairT[:, (c - 6) * 128:(c - 6 + 1) * 128]
            nc.tensor.matmul(fin[:], lhsT, wout_sb[:, c * C_S:(c + 1) * C_S],
                             start=(idx == 0), stop=False)
        nc.tensor.matmul(fin[:], ones1[:], bout_sb[:], start=False, stop=True)
        out_sb = blkp.tile([128, C_S], F32, tag="out_sb", name="out_sb")
        nc.vector.tensor_copy(out_sb[:], fin[:])
        dma(p['out'][:], out_sb[:])

    zpt_cm.__exit__(None, None, None)
    zpn_cm.__exit__(None, None, None)
    pers_cm.__exit__(None, None, None)


# ======================= driver =======================
_NC_CACHE = {}


def _get_nc():
    if 'nc' not in _NC_CACHE:
        _NC_CACHE['nc'] = build_kernel()
    return _NC_CACHE['nc']


def kernel(**inputs):
    """Full-input IPA forward on 8 NeuronCores. Returns [B, N, C_S] float32."""
    from concourse.bass_utils import run_bass_kernel_spmd
    inp = {k: np.asarray(v) for k, v in inputs.items()}
    packed = pack_weights(inp)
    in_maps, meta = [], []
    for core in range(N_CORES):
        m, b, i0 = per_core_inputs(inp, packed, core)
        in_maps.append(m)
        meta.append((b, i0))
    nc = _get_nc()
    res = run_bass_kernel_spmd(nc, in_maps, core_ids=list(range(N_CORES)))
    out = np.zeros((B, N, C_S), np.float32)
    for core in range(N_CORES):
        b, i0 = meta[core]
        out[b, i0:i0 + NI] = res.results[core]["out"]
    return out


# revision 58
# speedup vs baseline: 1.4668x; 1.4668x over previous
"""Invariant Point Attention (IPA) forward as a Bass/Tile kernel on 8
Trainium2 NeuronCores (SPMD, query-axis sharded).

Strategy: each core owns 128 query rows of one batch. Attention logits are a
single 384-feature f32r matmul per 4-query block (QK, point-attention cross
term, and -0.5*hw*||kg||^2 folded into one contraction); the pair bias z@wb
accumulates directly into the logits PSUM via per-query bf16 matmuls. z is
shipped from the host in bf16 in TWO packed layouts — natural (j on
partitions, for out_pair) and pre-transposed (d on partitions, for the pair
bias) — so no on-chip transposes or PSUM-drain copies of z are needed and z
HBM traffic is halved.
"""
import sys
sys.path.insert(0, '/opt/trn_rl_repo')

import math
import numpy as np

import concourse.bass as bass
import concourse.tile as tile
from concourse import bacc, mybir
from concourse.masks import make_identity

C_S, C_Z = 384, 128
H, D = 12, 16
PQ, PV = 4, 8
B, N = 2, 512
OUT_IN = H * D + H * PV * 4 + H * C_Z  # 2112
EPS = 1e-8

N_CORES = 8
NI = 128            # query rows per core
IBLK = 4            # queries per block (each padded to 32 partitions)
NBLK = NI // IBLK   # 32 blocks
SBLK = 8            # blocks per superblock (32 i's)
NSUP = NBLK // SBLK  # 4 superblocks

SCALE_QK = math.sqrt(1.0 / (3.0 * D))
SCALE_B = math.sqrt(1.0 / 3.0)
WC = math.sqrt(1.0 / (3.0 * (PQ * 9.0 / 2.0)))

# ---- proj feature column order for the fused projection s @ Wcat ----
#   A 0:192     k    (h*16+dd)
#   B 192:384   v    (h*16+dd)
#   C 384:528   kp   (c*48 + h*4 + q)
#   D 528:816   vp   (c*96 + h*8 + pv)
#   E 816:1008  q    (h*16+dd)
#   F 1008:1152 qp   (c*48 + h*4 + q)
PROJ = 1152
# host packing offsets (pre-permutation)
OFF_K, OFF_V, OFF_KP, OFF_VP, OFF_Q, OFF_QP = 0, 192, 384, 528, 816, 1008
# device column order after permutation: K | KP | V | VP | Q | QP
DOFF_K, DOFF_KP, DOFF_V, DOFF_VP, DOFF_Q, DOFF_QP = 0, 192, 336, 528, 816, 1008

# kfullT / fT_q global row map (384 rows = 3 chunks of 128):
#   0:192    k rows   (h*16+dd)      [chunk0 0:128 | chunk1 0:64]
#   192:336  kg rows  (c*48+h*4+q)   [chunk1 64:128 | chunk2 0:64 | chunk2 64:80]
#   352:364  C / E rows (h)          [chunk2 96:108]
KF_PAD = 384


def softplus(x):
    return np.logaddexp(0.0, x)


def pack_weights(inp):
    """Host-side packing of the small weight tensors (layout-only + tiny
    scalar math). Returns dict of arrays shared by all cores."""
    wq, bq = np.asarray(inp['wq']), np.asarray(inp['bq'])
    wkv, bkv = np.asarray(inp['wkv']), np.asarray(inp['bkv'])
    wqp, bqp = np.asarray(inp['wqp']), np.asarray(inp['bqp'])
    wkvp, bkvp = np.asarray(inp['wkvp']), np.asarray(inp['bkvp'])
    wb = np.asarray(inp['wb'])
    wout, bout = np.asarray(inp['wout']), np.asarray(inp['bout'])
    head_weights = np.asarray(inp['head_weights'])

    wcat = np.zeros((C_S, PROJ), np.float32)
    bcat = np.zeros((PROJ,), np.float32)

    kv = wkv.reshape(C_S, H, 2 * D)
    bkv2 = bkv.reshape(H, 2 * D)
    wcat[:, OFF_K:OFF_K + 192] = kv[:, :, :D].reshape(C_S, 192)
    bcat[OFF_K:OFF_K + 192] = bkv2[:, :D].reshape(192)
    wcat[:, OFF_V:OFF_V + 192] = kv[:, :, D:].reshape(C_S, 192)
    bcat[OFF_V:OFF_V + 192] = bkv2[:, D:].reshape(192)

    kvp = wkvp.reshape(C_S, 3, H, PQ + PV)
    bkvp2 = bkvp.reshape(3, H, PQ + PV)
    wcat[:, OFF_KP:OFF_KP + 144] = kvp[:, :, :, :PQ].reshape(C_S, 144)
    bcat[OFF_KP:OFF_KP + 144] = bkvp2[:, :, :PQ].reshape(144)
    wcat[:, OFF_VP:OFF_VP + 288] = kvp[:, :, :, PQ:].reshape(C_S, 288)
    bcat[OFF_VP:OFF_VP + 288] = bkvp2[:, :, PQ:].reshape(288)
    # device order: K | KP | V | VP | Q | QP  (V+VP contiguous -> one
    # proj section draining to a persistent tile, transformed in-loop)
    perm_cols = (list(range(0, 192)) + list(range(384, 528))
                 + list(range(192, 384)) + list(range(528, 816))
                 + list(range(816, 1152)))
    wcat = wcat[:, perm_cols]
    bcat = bcat[perm_cols]

    wcat[:, OFF_Q:OFF_Q + 192] = wq
    bcat[OFF_Q:OFF_Q + 192] = bq
    wcat[:, OFF_QP:OFF_QP + 144] = wqp
    bcat[OFF_QP:OFF_QP + 144] = bqp

    import ml_dtypes as _mld
    wcat_dev = wcat.reshape(3, 128, PROJ).astype(_mld.bfloat16)

    hw = (softplus(head_weights) * WC).astype(np.float32)          # [H]

    # wbs: wb * SCALE_B (bb is constant over j -> softmax-invariant, dropped)
    wbs = (wb * SCALE_B).astype(np.float32)

    # qsel [3, 128, H]: row-selector+scale for building qexp from fT_q.
    qsel = np.zeros((3, 128, H), np.float32)
    for h in range(H):
        for dd in range(D):
            g = h * D + dd           # k rows
            qsel[g // 128, g % 128, h] = SCALE_QK
    for c in range(3):
        for h in range(H):
            for q in range(PQ):
                g = c * 48 + h * 4 + q   # kg feature index 0..144
                if g < 64:
                    ch, r = 1, 64 + g
                elif g < 128:
                    ch, r = 2, g - 64
                else:
                    ch, r = 2, 64 + (g - 128)
                qsel[ch, r, h] = hw[h]
    for h in range(H):
        qsel[2, 96 + h, h] = -0.5 * hw[h]   # E rows

    # wout reordered+padded to featsT row order, 18 chunks of 128 (bf16):
    import ml_dtypes
    wout_dev = np.zeros((18, 128, C_S), ml_dtypes.bfloat16)
    wo = wout.astype(ml_dtypes.bfloat16)  # [2112, 384]
    wout_dev[0] = wo[0:128]
    wout_dev[1, :64] = wo[128:192]
    wout_dev[2, :96] = wo[192:288]      # rpl_x
    wout_dev[3, :96] = wo[288:384]      # rpl_y
    wout_dev[4, :96] = wo[384:480]      # rpl_z
    wout_dev[5, :96] = wo[480:576]      # norms
    for h in range(H):
        wout_dev[6 + h] = wo[576 + h * C_Z: 576 + (h + 1) * C_Z]

    return dict(
        wcat_dev=wcat_dev, bcat=bcat.reshape(1, PROJ), wbs=wbs,
        qsel=qsel, wout_dev=wout_dev,
        bout=bout.reshape(1, C_S).astype(np.float32), hw=hw,
    )


def per_core_inputs(inp, packed, core):
    """Build the input map for one core.

    The j (key/residue) axis is rolled per core so this core's 128 query
    rows sit at j'=0..127 — softmax/attention sums over j are order
    invariant, and the query-side projections/frames become the jc=0 slice
    of the key-side ones (no separate sTq/Rtq stream needed).
    """
    b = core // 4
    i0 = (core % 4) * NI
    s = np.asarray(inp['single_representation'])[b]          # [512, 384]
    z = np.asarray(inp['pair_representation'])[b]            # [512, 512, 128]
    R = np.asarray(inp['rotations'])[b]                      # [512, 3, 3]
    t = np.asarray(inp['translation'])[b]                    # [512, 3]

    import ml_dtypes
    perm = (np.arange(N) + i0) % N
    s = s[perm]
    R = R[perm]
    t = t[perm]

    sT = np.ascontiguousarray(s.T.reshape(3, 128, N).astype(ml_dtypes.bfloat16))
    Rt = np.concatenate([R.reshape(N, 9), t], axis=1).astype(np.float32)  # [512, 12]

    # z slab for our 128 queries (j rolled), bf16, two packed layouts:
    #   znat[bk, p, il, jc, d] = z'[bk*4+il, jc*128+p, d]   (j on partitions)
    #   ztr [bk, d, il, j]     = z'[bk*4+il, j, d]          (d on partitions)
    zq = z[i0:i0 + NI][:, perm].astype(ml_dtypes.bfloat16)   # [128, 512, 128]
    z5 = zq.reshape(NBLK, IBLK, 4, 128, C_Z)                 # [bk, il, jc, p, d]
    znat = np.ascontiguousarray(z5.transpose(0, 3, 1, 2, 4)).reshape(NBLK, 128, IBLK * 4 * C_Z)
    # bias path is softmax-smoothed: fp8 is enough and halves its HBM bytes
    zq8 = z[i0:i0 + NI][:, perm].astype(ml_dtypes.float8_e4m3fn)
    ztr = np.ascontiguousarray(zq8.reshape(NBLK, IBLK, N, C_Z).transpose(0, 3, 1, 2)).reshape(NBLK, C_Z, IBLK * N)

    return {
        'sT': sT,
        'znat': znat, 'ztr': ztr,
        'Rt': np.ascontiguousarray(Rt),
        'wcat_dev': packed['wcat_dev'], 'bcat': packed['bcat'],
        'wbs': packed['wbs'], 'qsel': packed['qsel'],
        'wout_dev': packed['wout_dev'], 'bout': packed['bout'],
    }, b, i0


F32 = mybir.dt.float32
F32R = mybir.dt.float32r
BF16 = mybir.dt.bfloat16
F8E4 = mybir.dt.float8e4
AF = mybir.ActivationFunctionType
ALU = mybir.AluOpType
AX = mybir.AxisListType


def r32(x):
    return x.bitcast(F32R)


def declare_params(nc):
    p = {}
    p['sT'] = nc.declare_dram_parameter("sT", [3, 128, N], BF16, isOutput=False)
    p['znat'] = nc.declare_dram_parameter("znat", [NBLK, 128, IBLK * 4 * C_Z], BF16, isOutput=False)
    p['ztr'] = nc.declare_dram_parameter("ztr", [NBLK, C_Z, IBLK * N], F8E4, isOutput=False)
    p['Rt'] = nc.declare_dram_parameter("Rt", [N, 12], F32, isOutput=False)
    p['wcat'] = nc.declare_dram_parameter("wcat_dev", [3, 128, PROJ], BF16, isOutput=False)
    p['bcat'] = nc.declare_dram_parameter("bcat", [1, PROJ], F32R, isOutput=False)
    p['wbs'] = nc.declare_dram_parameter("wbs", [C_Z, H], F32, isOutput=False)
    p['qsel'] = nc.declare_dram_parameter("qsel", [3, 128, H], F32, isOutput=False)
    p['wout'] = nc.declare_dram_parameter("wout_dev", [18, 128, C_S], BF16, isOutput=False)
    p['bout'] = nc.declare_dram_parameter("bout", [1, C_S], F32R, isOutput=False)
    p['out'] = nc.declare_dram_parameter("out", [NI, C_S], F32, isOutput=True)
    return p


def build_kernel(repeat=1):
    nc = bacc.Bacc("TRN2", target_bir_lowering=False, debug=False,
                   num_devices=N_CORES)
    p = declare_params(nc)

    with tile.TileContext(nc) as tc:
        if repeat > 1:
            with tc.For_i(0, repeat, 1):
                _body(nc, tc, p)
        else:
            _body(nc, tc, p)
    nc.compile()
    return nc


def _body(nc, tc, p):
    dma = nc.sync.dma_start

    pers_cm = tc.tile_pool(name="pers", bufs=1)
    pers = pers_cm.__enter__()

    ident = pers.tile([128, 128], F32)
    make_identity(nc, ident[:])
    ident_r = pers.tile([128, 128], F32R)
    nc.vector.tensor_copy(ident_r[:], ident[:])
    ident_b = pers.tile([128, 128], BF16)
    nc.vector.tensor_copy(ident_b[:], ident[:])
    ones1 = pers.tile([1, 128], F32R)
    nc.gpsimd.memset(ones1[:].bitcast(F32), 1.0)

    wbs_bf = pers.tile([C_Z, H], F8E4)
    Rt_sb = pers.tile([128, 4 * 12], F32)
    dma(Rt_sb[:].rearrange("p (jc w) -> p jc w", jc=4),
        p['Rt'].rearrange("(jc p) w -> p jc w", jc=4))

    kfullT = [pers.tile([128, N], F32R, tag=f"kfullT{c}", name=f"kfullT{c}")
              for c in range(3)]
    qexp = [pers.tile([128, NI * 32], F32R, tag=f"qexp{c}", name=f"qexp{c}")
            for c in range(3)]
    # Only rows that stay junk after the phase-0 writes need zeroing (junk
    # could be NaN bit patterns; NaN*0 = NaN would poison the logits).
    nc.gpsimd.memset(kfullT[2][64:128, :].bitcast(F32), 0.0)
    vvg_bf = pers.tile([128, 4 * 480], BF16)
    pairT = pers.tile([128, H * 128], BF16)
    fts = pers.tile([128, 6 * 128], BF16)
    # fts rows never written per-chunk must be zero (chunk1 rows 64:128,
    # chunks 2..5 rows 96:128) so junk doesn't poison the final projection.
    nc.gpsimd.memset(fts[64:128, 128:256], 0.0)
    nc.gpsimd.memset(fts[96:128, 256:768], 0.0)
    sumsq = pers.tile([128, 96], F32)
    vraw = pers.tile([128, 4 * 480], F32)   # V|VP proj drains, per j-chunk
    wout_sb = pers.tile([128, 18 * C_S], BF16)
    bout_sb = pers.tile([1, C_S], F32R)
    epsb = pers.tile([128, 1], F32)
    nc.gpsimd.memset(epsb[:], EPS)

    # z streaming pools opened before phase-0 scratch so their SBUF does not
    # alias phase-0 tiles — z DMAs then overlap phase-0 compute. z_sb gets
    # extra depth: the deferred out_pair stage keeps it alive one block
    # longer than zt_sb.
    zpn_cm = tc.tile_pool(name="zpn", bufs=8)
    zpn = zpn_cm.__enter__()
    zpt_cm = tc.tile_pool(name="zpt", bufs=8)
    zpt = zpt_cm.__enter__()

    PREFETCH = 8
    z_tiles = {}

    def fetch_block(bk):
        z_sb = zpn.tile([128, IBLK * 4 * C_Z], BF16, tag="z", name="z_sb")
        dma(z_sb[:], p['znat'][bk])
        zt_sb = zpt.tile([C_Z, IBLK * N], F8E4, tag="zt", name="zt_sb")
        dma(zt_sb[:], p['ztr'][bk])
        z_tiles[bk] = (z_sb, zt_sb)

    # =================== PHASE 0 ===================
    with tc.tile_pool(name="ph0", bufs=1) as ph0, \
         tc.tile_pool(name="ph0b", bufs=2) as ph0b, \
         tc.tile_pool(name="ph0ps", bufs=2, space="PSUM") as ph0ps:
        # Weight/input DMAs split per chunk and interleaved so the first
        # proj matmuls can start as soon as chunk 0 lands.
        wcat_sb = ph0.tile([128, 3 * PROJ], BF16)
        sT_sb = ph0.tile([128, 3 * N], BF16)
        for kc in range(3):
            dma(wcat_sb[:, kc * PROJ:(kc + 1) * PROJ], p['wcat'][kc])
            dma(sT_sb[:, kc * N:(kc + 1) * N], p['sT'][kc])
        bcat_sb = ph0.tile([1, PROJ], F32R)
        dma(bcat_sb[:], p['bcat'][:])
        wbs_f = ph0.tile([C_Z, H], F32)
        dma(wbs_f[:], p['wbs'][:])
        nc.vector.tensor_copy(wbs_bf[:], wbs_f[:])
        qsel_sb = ph0.tile([128, 3 * H], F32)
        dma(qsel_sb[:].rearrange("p (c w) -> p c w", c=3),
            p['qsel'].rearrange("c p w -> p c w"))
        dma(bout_sb[:], p['bout'][:])

        # First blocks' z DMAs: after the phase-0 weight DMAs (which gate
        # compute) but before the phase-0 closing barrier, so they stream
        # while phase-0 computes.
        for bk in range(PREFETCH):
            fetch_block(bk)

        proj_nat = ph0.tile([128, 4 * PROJ], F32R)   # j-chunks; jc=0 = queries
        projq = proj_nat[:, 0:PROJ]
        kg_nat = ph0.tile([128, 4 * 144], F32R)
        qg_nat = ph0.tile([128, 144], F32R)
        c_nat = ph0.tile([128, 4 * 12], F32R)
        fTq = [ph0.tile([128, NI], F32R, tag=f"fTq{c}", name=f"fTq{c}")
               for c in range(3)]
        nc.gpsimd.memset(fTq[2][64:128, :].bitcast(F32), 0.0)
        nc.gpsimd.memset(fTq[2][96:108, :].bitcast(F32), 1.0)

        proj_rr = [0]

        def proj_jc(jc):
            # sections: S0 K|KP -> proj_nat, S1 V|VP -> persistent vraw,
            # S2 Q|QP (queries chunk only) -> proj_nat
            secs = [(0, 336, 'p'), (336, 816, 'v')]
            if jc == 0:
                secs.append((816, 1152, 'p'))
            for (w0, w1, kind) in secs:
                wd = w1 - w0
                pp = ph0ps.tile([128, 480], F32, tag="projps", name="pp")
                for kc in range(3):
                    nc.tensor.matmul(
                        pp[:, 0:wd],
                        sT_sb[:, kc * N + jc * 128: kc * N + (jc + 1) * 128],
                        wcat_sb[:, kc * PROJ + w0: kc * PROJ + w1],
                        start=(kc == 0), stop=False)
                nc.tensor.matmul(pp[:, 0:wd], ones1[:], bcat_sb[:, w0:w1],
                                 start=False, stop=True)
                eng = proj_rr[0] % 2
                proj_rr[0] += 1
                if kind == 'v':
                    dst = vraw[:, jc * 480:(jc + 1) * 480]
                else:
                    dcol = jc * PROJ + w0
                    dst = proj_nat[:, dcol:dcol + wd]
                if eng == 0:
                    nc.vector.tensor_copy(dst, pp[:, 0:wd])
                else:
                    nc.scalar.copy(dst, pp[:, 0:wd])

        def glob_points(dst, dcol0, src, scol0, bw, n_rt, eng=None):
            eng = eng or nc.vector
            for cp in range(3):
                d = dst[:, dcol0 + cp * bw: dcol0 + (cp + 1) * bw]
                eng.tensor_scalar(
                    d, src[:, scol0:scol0 + bw].bitcast(F32),
                    Rt_sb[:, n_rt + cp * 3: n_rt + cp * 3 + 1],
                    Rt_sb[:, n_rt + 9 + cp: n_rt + 9 + cp + 1],
                    op0=ALU.mult, op1=ALU.add)
                for pp_ in (1, 2):
                    eng.scalar_tensor_tensor(
                        d, src[:, scol0 + pp_ * bw: scol0 + (pp_ + 1) * bw].bitcast(F32),
                        Rt_sb[:, n_rt + cp * 3 + pp_: n_rt + cp * 3 + pp_ + 1],
                        d.bitcast(F32),
                        op0=ALU.mult, op1=ALU.add)

        def cterm_jc(jc):
            sq = ph0b.tile([128, 144], F32, tag="sq", name="sq")
            nc.scalar.activation(sq[:], kg_nat[:, jc * 144:(jc + 1) * 144].bitcast(F32), AF.Square)
            red1 = ph0b.tile([128, 12 * 3], F32, tag="red1", name="red1")
            nc.vector.tensor_reduce(
                red1[:].rearrange("p (h c) -> p h c", h=H),
                sq[:].rearrange("p (c h q) -> p h c q", c=3, h=H),
                axis=AX.X, op=ALU.add)
            with nc.allow_low_precision(reason="f32r rounding of Cterm is fine"):
                nc.vector.tensor_reduce(
                    c_nat[:, jc * 12:(jc + 1) * 12].unsqueeze(-1),
                    red1[:].rearrange("p (h c) -> p h c", h=H),
                    axis=AX.X, op=ALU.add)

        # PSUM drains round-robin DVE/Pool/Act — phase 0 is DVE-bound.
        pe_T_rr = [0]

        def pe_T(dst_col_writes, src_ap):
            tp = ph0ps.tile([128, 128], F32R, tag="tps", name="tp")
            np_ = src_ap.shape[0]
            nf = src_ap.shape[1]
            nc.tensor.transpose(tp[0:nf, 0:np_], r32(src_ap),
                                ident_r[0:np_, 0:np_])
            for (dst, r0, r1) in dst_col_writes:
                eng = pe_T_rr[0] % 2
                pe_T_rr[0] += 1
                if r0 % 64 != 0 or eng:
                    nc.scalar.copy(dst, tp[r0:r1, 0:np_].bitcast(F32))
                else:
                    nc.vector.tensor_copy(dst, tp[r0:r1, 0:np_].bitcast(F32))

        def transposes_jc(jc):
            js = slice(jc * 128, (jc + 1) * 128)
            pe_T([(kfullT[0][0:128, js], 0, 128)],
                 proj_nat[:, jc * PROJ + DOFF_K: jc * PROJ + DOFF_K + 128])
            pe_T([(kfullT[1][0:64, js], 0, 64)],
                 proj_nat[:, jc * PROJ + DOFF_K + 128: jc * PROJ + DOFF_K + 192])
            pe_T([(kfullT[1][64:128, js], 0, 64), (kfullT[2][0:64, js], 64, 128)],
                 kg_nat[:, jc * 144: jc * 144 + 128])
            pe_T([(kfullT[2][64:80, js], 0, 16)],
                 kg_nat[:, jc * 144 + 128: jc * 144 + 144])
            pe_T([(kfullT[2][96:108, js], 0, 12)], c_nat[:, jc * 12:(jc + 1) * 12])

        # Critical-path order: everything the first block needs (proj, kg,
        # Cterm, kfullT, fTq, qexp), then the epilogue-only vg/vvg.
        for jc in range(4):
            proj_jc(jc)
            glob_points(kg_nat, jc * 144, proj_nat, jc * PROJ + DOFF_KP, 48,
                        jc * 12)
            cterm_jc(jc)
            if jc == 0:
                glob_points(qg_nat, 0, projq, DOFF_QP, 48, 0)
            transposes_jc(jc)
            if jc == 0:
                pe_T([(fTq[0][0:128, :], 0, 128)], projq[:, DOFF_Q:DOFF_Q + 128])
                pe_T([(fTq[1][0:64, :], 0, 64)], projq[:, DOFF_Q + 128:DOFF_Q + 192])
                pe_T([(fTq[1][64:128, :], 0, 64), (fTq[2][0:64, :], 64, 128)],
                     qg_nat[:, 0:128])
                pe_T([(fTq[2][64:80, :], 0, 16)], qg_nat[:, 128:144])
                # qexp: per-head masked scale of fT_q. Only e<12 columns are
                # written; e>=12 stay junk — junk lg rows exp to junk attn
                # rows/attnT cols that nothing downstream reads.
                for c in range(3):
                    for h in range(H):
                        dst = qexp[c][:].rearrange("p (i e) -> p i e", e=32)[:, :, h]
                        args = (fTq[c][:].bitcast(F32),
                                qsel_sb[:, c * H + h: c * H + h + 1])
                        if h % 2 == 0:
                            nc.vector.tensor_scalar(dst, args[0], args[1], None,
                                                    op0=ALU.mult)
                        else:
                            nc.scalar.mul(dst, args[0], args[1])


    # =================== MAIN LOOP ===================
    # Software-pipelined one block deep: block bk's post-softmax stage
    # (attnT, PSUM drains, out_pair) is emitted during block bk+1, after
    # bk+1's logits matmuls — so the in-order PE queue never stalls on the
    # softmax chain with useful work queued behind it.
    with tc.tile_pool(name="blk", bufs=2) as blkp, \
         tc.tile_pool(name="sblk", bufs=2) as sblkp, \
         tc.tile_pool(name="ps_lg", bufs=2, space="PSUM") as ps_lg, \
         tc.tile_pool(name="ps_at", bufs=1, space="PSUM") as ps_at, \
         tc.tile_pool(name="ps_op", bufs=1, space="PSUM") as ps_op, \
         tc.tile_pool(name="ps_ep", bufs=1, space="PSUM") as ps_ep, \
         tc.tile_pool(name="ps_fin", bufs=1, space="PSUM") as ps_fin:
        attnTs = {}
        deferred = {}

        def vvg_build():
            # V|VP -> global-frame vg -> vvg_bf, off the phase-0 barrier:
            # runs during the first blocks, needed first at sb 0's epilogue.
            for jc in range(4):
                vg = blkp.tile([128, 288], F32, tag="vg", name="vg")
                glob_points(vg, 0, vraw, jc * 480 + 192, 96, jc * 12)
                nc.scalar.copy(
                    vvg_bf[:, jc * 480:(jc + 1) * 480]
                    .rearrange("p (h w) -> p h w", h=H)[:, :, 0:D],
                    vraw[:, jc * 480: jc * 480 + 192]
                    .rearrange("p (h d) -> p h d", h=H))
                nc.vector.tensor_copy(
                    vvg_bf[:, jc * 480:(jc + 1) * 480]
                    .rearrange("p (h w) -> p h w", h=H)[:, :, D:D + 24]
                    .rearrange("p h (c v) -> p c h v", c=3),
                    vg[:].rearrange("p (c h v) -> p c h v", c=3, h=H))

        def front_stage(bk):
            sb, lb = bk // SBLK, bk % SBLK
            if lb == 0:
                attnTs[sb] = sblkp.tile([128, SBLK * 4 * 128], BF16,
                                        tag="attnTb", name="attnT_b")
            z_sb, zt_sb = z_tiles.pop(bk)
            if bk + PREFETCH < NBLK:
                fetch_block(bk + PREFETCH)
            # wout is epilogue-only: stream it mid-loop in 3 slices so it
            # neither delays the first z blocks nor stalls the tail.
            if bk in (2, 6, 10):
                c0 = {2: 0, 6: 6, 10: 12}[bk]
                dma(wout_sb[:, c0 * C_S:(c0 + 6) * C_S]
                    .rearrange("p (c w) -> p c w", c=6),
                    p['wout'][c0:c0 + 6].rearrange("c p w -> p c w"))

            lg = ps_lg.tile([128, N], F32, tag="lg", name="lg")
            for kc in range(3):
                nc.tensor.matmul(
                    lg[:], r32(qexp[kc][:, bk * 128:(bk + 1) * 128]),
                    r32(kfullT[kc][:]), start=(kc == 0), stop=(kc == 2))
            for il in range(IBLK):
                nc.tensor.matmul(lg[32 * il:32 * il + H, :], wbs_bf[:],
                                 zt_sb[:, il * N:(il + 1) * N],
                                 start=False, stop=True,
                                 skip_group_check=True,
                                 tile_position=(0, 32 * il))

            # softmax (exp + sum + normalize), bf16 weights
            attn = blkp.tile([128, N], BF16, tag="attn", name="attn")
            ssum = blkp.tile([128, 1], F32, tag="ssum", name="ssum")
            nc.scalar.activation(attn[:], lg[:], AF.Exp, accum_out=ssum[:])
            rcp = blkp.tile([128, 1], F32, tag="rcp", name="rcp")
            nc.vector.reciprocal(rcp[:], ssum[:])
            nc.vector.tensor_scalar(attn[:], attn[:], rcp[:], None,
                                    op0=ALU.mult)
            if bk == NBLK - 1:
                # Anchor a tiny Sqrt right after the last Exp so the sqrt
                # act-table load overlaps the drain instead of the tail.
                warm = blkp.tile([1, 1], F32, tag="warm", name="warm")
                nc.scalar.activation(warm[:], ssum[0:1, :], AF.Sqrt)
            deferred[bk] = (z_sb, attn)

        def back_stage(bk):
            sb, lb = bk // SBLK, bk % SBLK
            z_sb, attn = deferred.pop(bk)
            attnT_b = attnTs[sb]
            # attnT -> bf16 slab (epilogue lhsT + out_pair rhs);
            # PSUM drain split Pool/DVE so neither is on the whole chain.
            atp = ps_at.tile([128, N], BF16, tag="atp", name="atp")
            for jc in range(4):
                nc.tensor.transpose(atp[:, jc * 128:(jc + 1) * 128],
                                    attn[:, jc * 128:(jc + 1) * 128],
                                    ident_b[:])
            at_dst = attnT_b[:].rearrange("p (jc l) -> p jc l", jc=4)
            nc.vector.tensor_copy(
                at_dst[:, :, lb * 128:(lb + 1) * 128],
                atp[:].rearrange("p (jc e) -> p jc e", jc=4))

            # out_pair, all 4 queries batched into one PSUM tile + 1 copy
            op_ps = ps_op.tile([128, IBLK * H], F32, tag="opps", name="op_ps")
            for il in range(IBLK):
                for jc in range(4):
                    nc.tensor.matmul(
                        op_ps[:, il * H:(il + 1) * H],
                        z_sb[:, (il * 4 + jc) * C_Z:(il * 4 + jc + 1) * C_Z],
                        attnT_b[:, jc * (SBLK * 128) + lb * 128 + il * 32:
                                jc * (SBLK * 128) + lb * 128 + il * 32 + H],
                        start=(jc == 0), stop=(jc == 3),
                        skip_group_check=True)
            dst = (pairT[:].rearrange("p (h i) -> p h i", h=H)
                   [:, :, bk * IBLK:(bk + 1) * IBLK])
            src = op_ps[:].rearrange("p (il h) -> p h il", il=IBLK)
            if lb % 2 == 0:
                nc.vector.tensor_copy(dst, src)
            else:
                nc.scalar.copy(dst, src)

        def sb_epilogue(sb):
            attnT_b = attnTs.pop(sb)
            r0 = sb * 32
            epi = ps_ep.tile([32, H * 40], F32, tag="epi", name="epi")
            at4 = attnT_b[:].rearrange("p (jc gil e) -> p jc gil e",
                                       jc=4, e=32)
            for h in range(H):
                for jc in range(4):
                    nc.tensor.matmul(
                        epi[:, h * 40:(h + 1) * 40],
                        at4[:, jc, :, h],
                        vvg_bf[:, jc * 480 + h * 40: jc * 480 + (h + 1) * 40],
                        start=(jc == 0), stop=(jc == 3))
            scal_sb = blkp.tile([32, H * D], F32R, tag="scal", name="scal_sb")
            rpg_sb = blkp.tile([32, 3 * 96], F32, tag="rpg", name="rpg_sb")
            nc.scalar.copy(
                scal_sb[:].rearrange("p (h d) -> p h d", h=H),
                epi[:].rearrange("p (h w) -> p h w", h=H)[:, :, 0:D])
            nc.vector.tensor_copy(
                rpg_sb[:].rearrange("p (c h v) -> p c h v", c=3, h=H),
                epi[:].rearrange("p (h w) -> p h w", h=H)[:, :, D:40]
                .rearrange("p h (c v) -> p c h v", c=3))

            # ---- rpl rotation + sumsq + feature transposes, spread per sb ----
            rows = slice(r0, r0 + 32)
            # Frame scalars restaged at partition base 0 (scalar-ptr operands
            # must share the base partition of the tensor operand).
            rtq32 = blkp.tile([32, 12], F32, tag="rtq32", name="rtq32")
            nc.scalar.copy(rtq32[:], Rt_sb[rows, 0:12])
            rpgm = blkp.tile([32, 3 * 96], F32, tag="rpgm", name="rpgm")
            for pp_ in range(3):
                nc.vector.tensor_scalar(rpgm[:, pp_ * 96:(pp_ + 1) * 96],
                                        rpg_sb[:, pp_ * 96:(pp_ + 1) * 96],
                                        rtq32[:, 9 + pp_: 9 + pp_ + 1], None,
                                        op0=ALU.subtract)
            rpl = blkp.tile([32, 3 * 96], F32R, tag="rpl", name="rpl")
            for o in range(3):
                eng = nc.vector
                d = rpl[:, o * 96:(o + 1) * 96]
                eng.tensor_scalar(d, rpgm[:, 0:96], rtq32[:, o:o + 1], None,
                                  op0=ALU.mult)
                for pp_ in (1, 2):
                    eng.scalar_tensor_tensor(
                        d, rpgm[:, pp_ * 96:(pp_ + 1) * 96],
                        rtq32[:, pp_ * 3 + o: pp_ * 3 + o + 1], d.bitcast(F32),
                        op0=ALU.mult, op1=ALU.add)
            sq2 = blkp.tile([32, 3 * 96], F32, tag="sq2", name="sq2")
            nc.scalar.activation(sq2[:], rpl[:].bitcast(F32), AF.Square)
            nsq = blkp.tile([32, 96], F32, tag="nsq", name="nsq")
            nc.vector.tensor_tensor(nsq[:], sq2[:, 0:96], sq2[:, 96:192], op=ALU.add)
            nc.vector.tensor_tensor(nsq[:], nsq[:], sq2[:, 192:288], op=ALU.add)
            nc.scalar.copy(sumsq[rows, :], nsq[:])

            t2rr = [0]

            def pe_T2(dst, src_ap, nrows):
                tp2 = ps_ep.tile([128, 32], F32R, tag="tps2", name="tp2")
                nc.tensor.transpose(tp2[0:nrows, :], r32(src_ap), ident_r[0:32, 0:32])
                t2rr[0] += 1
                if t2rr[0] % 2:
                    nc.vector.tensor_copy(dst, tp2[0:nrows, :].bitcast(F32))
                else:
                    nc.scalar.copy(dst, tp2[0:nrows, :].bitcast(F32))

            pe_T2(fts[0:128, r0:r0 + 32], scal_sb[:, 0:128], 128)
            pe_T2(fts[0:64, 128 + r0:128 + r0 + 32], scal_sb[:, 128:192], 64)
            for o in range(3):
                pe_T2(fts[0:96, (2 + o) * 128 + r0:(2 + o) * 128 + r0 + 32],
                      rpl[:, o * 96:(o + 1) * 96], 96)

            if sb == NSUP - 1:
                # norms = sqrt(sumsq + eps); Sqrt switches the act table once,
                # after the last Exp.
                nrm = blkp.tile([128, 96], F32R, tag="nrm", name="nrm")
                nc.scalar.activation(nrm[:], sumsq[:], AF.Sqrt, bias=epsb[:])
                tpn = ps_ep.tile([128, 128], F32R, tag="tpn", name="tpn")
                nc.tensor.transpose(tpn[0:96, :], nrm[:], ident_r[:])
                nc.vector.tensor_copy(fts[0:96, 5 * 128:6 * 128],
                                      tpn[0:96, :].bitcast(F32))

        for bk in range(NBLK):
            front_stage(bk)
            if bk == 1:
                vvg_build()
            if bk > 0:
                back_stage(bk - 1)
            if bk % SBLK == 0 and bk > 0:
                sb_epilogue(bk // SBLK - 1)
        back_stage(NBLK - 1)
        sb_epilogue(NSUP - 1)

        # ---- final projection, inside the pool scope (no close barrier) ----
        fin = ps_fin.tile([128, C_S], F32, tag="fin", name="fin")
        fin_order = list(range(6, 18)) + [0, 1, 2, 3, 4, 5]
        for idx, c in enumerate(fin_order):
            lhsT = fts[:, c * 128:(c + 1) * 128] if c < 6 else \
                pairT[:, (c - 6) * 128:(c - 6 + 1) * 128]
            nc.tensor.matmul(fin[:], lhsT, wout_sb[:, c * C_S:(c + 1) * C_S],
                             start=(idx == 0), stop=False)
        nc.tensor.matmul(fin[:], ones1[:], bout_sb[:], start=False, stop=True)
        out_sb = blkp.tile([128, C_S], F32, tag="out_sb", name="out_sb")
        nc.vector.tensor_copy(out_sb[:], fin[:])
        dma(p['out'][:], out_sb[:])

    zpt_cm.__exit__(None, None, None)
    zpn_cm.__exit__(None, None, None)
    pers_cm.__exit__(None, None, None)


# ======================= driver =======================
_NC_CACHE = {}


def _get_nc():
    if 'nc' not in _NC_CACHE:
        _NC_CACHE['nc'] = build_kernel()
    return _NC_CACHE['nc']


def kernel(**inputs):
    """Full-input IPA forward on 8 NeuronCores. Returns [B, N, C_S] float32."""
    from concourse.bass_utils import run_bass_kernel_spmd
    inp = {k: np.asarray(v) for k, v in inputs.items()}
    packed = pack_weights(inp)
    in_maps, meta = [], []
    for core in range(N_CORES):
        m, b, i0 = per_core_inputs(inp, packed, core)
        in_maps.append(m)
        meta.append((b, i0))
    nc = _get_nc()
    res = run_bass_kernel_spmd(nc, in_maps, core_ids=list(range(N_CORES)))
    out = np.zeros((B, N, C_S), np.float32)
    for core in range(N_CORES):
        b, i0 = meta[core]
        out[b, i0:i0 + NI] = res.results[core]["out"]
    return out
